# revision 5
# baseline (speedup 1.0000x reference)
import os
import sys

sys.path.insert(0, "/opt/trn_rl_repo")
import numpy as np
import ml_dtypes

import concourse.bass as bass
import concourse.tile as tile
import concourse.bacc as bacc
from concourse import mybir
from concourse.bass_utils import run_bass_kernel_spmd
from contextlib import ExitStack

BF16 = mybir.dt.bfloat16
F32 = mybir.dt.float32
F8 = mybir.dt.float8e4
DR = mybir.MatmulPerfMode.DoubleRow
AF = mybir.ActivationFunctionType
ds = bass.ds
ts = bass.ts

N_CORES = 8
EMBED = 768
BLOCKS = 8
BS = 96
LATENT = 3072
LAMBD = 0.01
EPS = 1e-5
H = 128
W = 128
WF = 65
SPEC = H * WF          # 8320 spectral pixels
SPX = SPEC // N_CORES  # 1040 per core
SPX2 = 2 * SPX         # 2080 merged re|im stream
FCH = 416              # f-conv pixel chunk (5 per core)
PX = (H * W) // N_CORES  # 2048 spatial pixels per core
MS = 64.0              # fp8 weight prescale (conv1s)
MS2 = 64.0             # fp8 weight prescale (conv2s)
FP8_F = False          # f-conv in fp8 (else bf16); bf16 keeps rel err ~5e-3
USE_CC = True          # TP-shard big weights + on-device AllGather (8x less transfer)

LAST_RESULTS = None
_PROG1 = None
_PROG2 = None
_WCACHE = {}


# ---------------------------------------------------------------- L1 program
def _build_l1():
    global _PROG1
    if _PROG1 is not None:
        return _PROG1
    nc = bacc.Bacc("TRN2", target_bir_lowering=False, debug=False, num_devices=N_CORES)
    FDT = F8 if FP8_F else BF16

    fa = nc.dram_tensor("fa", [EMBED, SPX2], FDT, kind="ExternalInput")
    if USE_CC:
        fw1s = nc.dram_tensor("fw1s", [3, 6, 128, 128], FDT, kind="ExternalInput")
        fw2s = nc.dram_tensor("fw2s", [3, 12, 128, 128], FDT, kind="ExternalInput")
    else:
        fw1 = nc.dram_tensor("fw1s", [24, 6, 128, 128], FDT, kind="ExternalInput")
        fw2g = nc.dram_tensor("fw2s", [24, 12, 128, 128], FDT, kind="ExternalInput")
    fb1 = nc.dram_tensor("fb1", [4 * EMBED, 1], F32, kind="ExternalInput")
    fb2 = nc.dram_tensor("fb2", [2 * EMBED, 1], F32, kind="ExternalInput")
    xr = nc.dram_tensor("xr", [BLOCKS, 128, SPX], BF16, kind="ExternalInput")
    xi = nc.dram_tensor("xi", [BLOCKS, 128, SPX], BF16, kind="ExternalInput")
    bw = nc.dram_tensor("bw", [BLOCKS, 6, 128, BS], BF16, kind="ExternalInput")
    bb1r = nc.dram_tensor("bb1r", [EMBED, 1], F32, kind="ExternalInput")
    bb1i = nc.dram_tensor("bb1i", [EMBED, 1], F32, kind="ExternalInput")
    bb2r = nc.dram_tensor("bb2r", [EMBED, 1], F32, kind="ExternalInput")
    bb2i = nc.dram_tensor("bb2i", [EMBED, 1], F32, kind="ExternalInput")
    o2r = nc.dram_tensor("o2r", [EMBED, SPX], BF16, kind="ExternalOutput")
    o2i = nc.dram_tensor("o2i", [EMBED, SPX], BF16, kind="ExternalOutput")
    ssf = nc.dram_tensor("ssf", [2 * EMBED, SPX2], BF16, kind="Internal")

    GROUPS = [list(range(N_CORES))]
    BYP = mybir.AluOpType.bypass
    with tile.TileContext(nc) as tc:
        # ---- stage 0: all-gather TP-sharded filter weights ----
        s0 = ExitStack()
        if USE_CC:
            dpool = s0.enter_context(tc.tile_pool(name="wg_dram", bufs=1, space="DRAM"))
            fw1b = dpool.tile([3, 6, 128, 128], FDT)
            nc.sync.dma_start(fw1b[:], fw1s[:])
            fw1 = dpool.tile([24, 6, 128, 128], FDT, addr_space="Shared")
            nc.gpsimd.collective_compute("AllGather", BYP, replica_groups=GROUPS,
                                         ins=[fw1b.opt()], outs=[fw1.opt()])
            fw2b = dpool.tile([3, 12, 128, 128], FDT)
            nc.sync.dma_start(fw2b[:], fw2s[:])
            fw2g = dpool.tile([24, 12, 128, 128], FDT, addr_space="Shared")
            nc.gpsimd.collective_compute("AllGather", BYP, replica_groups=GROUPS,
                                         ins=[fw2b.opt()], outs=[fw2g.opt()])

        # ---- stage F: filter ss-CNN on merged [mr|mi] stream ----
        fctx = ExitStack()
        ap = fctx.enter_context(tc.tile_pool(name="f_a", bufs=1))
        wp = fctx.enter_context(tc.tile_pool(name="f_w", bufs=3))
        hp = fctx.enter_context(tc.tile_pool(name="f_h", bufs=1))
        sp = fctx.enter_context(tc.tile_pool(name="f_s", bufs=3))
        bp = fctx.enter_context(tc.tile_pool(name="f_b", bufs=3))
        pp = fctx.enter_context(tc.tile_pool(name="f_p", bufs=4, space="PSUM"))

        fat = ap.tile([128, 6, SPX2], FDT)
        nc.sync.dma_start(fat[:], fa.rearrange("(c p) s -> p c s", p=128))
        h1f = hp.tile([128, 24, SPX2], FDT)
        for h in range(24):
            w1t = wp.tile([128, 6, 128], FDT, tag="fw1")
            nc.sync.dma_start(w1t[:], fw1[ds(h, 1)].rearrange("one c p m -> p (one c) m"))
            b1t = bp.tile([128, 1], F32, tag="fb1")
            nc.sync.dma_start(b1t[:], fb1[ds(h * 128, 128), :])
            for px in range(5):
                ps = pp.tile([128, FCH], F32, tag="ps1")
                if FP8_F:
                    for c in range(3):
                        nc.tensor.matmul(
                            ps[:], w1t[:, 2 * c:2 * c + 2, :],
                            fat[:, 2 * c:2 * c + 2, ts(px, FCH)],
                            start=(c == 0), stop=(c == 2), perf_mode=DR)
                else:
                    for c in range(6):
                        nc.tensor.matmul(
                            ps[:], w1t[:, c, :], fat[:, c, ts(px, FCH)],
                            start=(c == 0), stop=(c == 5))
                nc.scalar.activation(h1f[:, h, ts(px, FCH)], ps[:], AF.Relu,
                                     bias=b1t[:, 0:1], scale=1.0 / MS)
        for o in range(12):
            w2t = wp.tile([128, 24, 128], FDT, tag="fw2")
            nc.sync.dma_start(w2t[:], fw2g[:, ds(o, 1), :, :].rearrange("k one p m -> p (k one) m"))
            b2t = bp.tile([128, 1], F32, tag="fb2")
            nc.sync.dma_start(b2t[:], fb2[ds(o * 128, 128), :])
            for px in range(5):
                ps = pp.tile([128, FCH], F32, tag="ps2")
                if FP8_F:
                    for k in range(12):
                        nc.tensor.matmul(
                            ps[:], w2t[:, 2 * k:2 * k + 2, :],
                            h1f[:, 2 * k:2 * k + 2, ts(px, FCH)],
                            start=(k == 0), stop=(k == 11), perf_mode=DR)
                else:
                    for k in range(24):
                        nc.tensor.matmul(
                            ps[:], w2t[:, k, :], h1f[:, k, ts(px, FCH)],
                            start=(k == 0), stop=(k == 23))
                sst = sp.tile([128, FCH], BF16, tag="sst")
                nc.scalar.activation(sst[:], ps[:], AF.Relu,
                                     bias=b2t[:, 0:1], scale=1.0 / MS2)
                nc.sync.dma_start(ssf[ds(o * 128, 128), ts(px, FCH)], sst[:])
        fctx.close()

        # ---- stage B: block-diagonal spectral mm + complex FiLM + softshrink ----
        bctx = ExitStack()
        bwp = bctx.enter_context(tc.tile_pool(name="b_w", bufs=2))
        bxp = bctx.enter_context(tc.tile_pool(name="b_x", bufs=2))
        bop = bctx.enter_context(tc.tile_pool(name="b_o", bufs=2))
        bsp = bctx.enter_context(tc.tile_pool(name="b_s", bufs=2))
        bbp = bctx.enter_context(tc.tile_pool(name="b_b", bufs=2))
        bpp = bctx.enter_context(tc.tile_pool(name="b_p", bufs=2, space="PSUM"))
        bcp = bctx.enter_context(tc.tile_pool(name="b_c", bufs=1))
        lam = bcp.tile([BS, 1], F32)
        nc.vector.memset(lam[:], -LAMBD)
        CHUNKS = [(0, 512), (512, 512), (1024, 16)]
        for b in range(8):
            wt = bwp.tile([128, 6, BS], BF16, tag="bw")
            nc.sync.dma_start(wt[:], bw[ds(b, 1)].rearrange("one w i o -> i (one w) o"))
            w1r, w1i, w1in = wt[:, 0, :], wt[:, 1, :], wt[:, 2, :]
            w2r, w2i, w2in = wt[:, 3, :], wt[:, 4, :], wt[:, 5, :]
            xrt = bxp.tile([128, SPX], BF16, tag="xr")
            nc.sync.dma_start(xrt[:], xr[ds(b, 1)].rearrange("one c s -> c (one s)"))
            xit = bxp.tile([128, SPX], BF16, tag="xi")
            nc.sync.dma_start(xit[:], xi[ds(b, 1)].rearrange("one c s -> c (one s)"))
            b1rt = bbp.tile([BS, 1], F32, tag="b1r")
            nc.sync.dma_start(b1rt[:], bb1r[ds(BS * b, BS), :])
            b1it = bbp.tile([BS, 1], F32, tag="b1i")
            nc.sync.dma_start(b1it[:], bb1i[ds(BS * b, BS), :])
            b2rt = bbp.tile([BS, 1], F32, tag="b2r")
            nc.sync.dma_start(b2rt[:], bb2r[ds(BS * b, BS), :])
            b2it = bbp.tile([BS, 1], F32, tag="b2i")
            nc.sync.dma_start(b2it[:], bb2i[ds(BS * b, BS), :])

            o1r = bop.tile([BS, SPX], F32, tag="o1r")
            o1i = bop.tile([BS, SPX], F32, tag="o1i")
            for st, ln in CHUNKS:
                psr = bpp.tile([BS, 512], F32, tag="psr")
                nc.tensor.matmul(psr[:, :ln], w1r, xrt[:, st:st + ln], start=True, stop=False)
                nc.tensor.matmul(psr[:, :ln], w1in, xit[:, st:st + ln], start=False, stop=True)
                nc.scalar.activation(o1r[:, st:st + ln], psr[:, :ln], AF.Identity,
                                     bias=b1rt[:, 0:1])
                psi = bpp.tile([BS, 512], F32, tag="psi")
                nc.tensor.matmul(psi[:, :ln], w1r, xit[:, st:st + ln], start=True, stop=False)
                nc.tensor.matmul(psi[:, :ln], w1i, xrt[:, st:st + ln], start=False, stop=True)
                nc.scalar.activation(o1i[:, st:st + ln], psi[:, :ln], AF.Identity,
                                     bias=b1it[:, 0:1])
            scr = bsp.tile([BS, SPX], BF16, tag="scr")
            nc.sync.dma_start(scr[:], ssf[ds(BS * b, BS), 0:SPX])
            sci = bsp.tile([BS, SPX], BF16, tag="sci")
            nc.sync.dma_start(sci[:], ssf[ds(BS * b, BS), SPX:SPX2])
            shr = bsp.tile([BS, SPX], BF16, tag="shr")
            nc.sync.dma_start(shr[:], ssf[ds(EMBED + BS * b, BS), 0:SPX])
            shi = bsp.tile([BS, SPX], BF16, tag="shi")
            nc.sync.dma_start(shi[:], ssf[ds(EMBED + BS * b, BS), SPX:SPX2])
            # n_re = o1r*(1+scr) - o1i*(1+sci) + shr
            # n_im = o1i*(1+scr) + o1r*(1+sci) + shi
            nr = bop.tile([BS, SPX], F32, tag="nr")
            t2 = bop.tile([BS, SPX], F32, tag="t2")
            nc.vector.scalar_tensor_tensor(nr[:], scr[:], 1.0, o1r[:],
                                           mybir.AluOpType.add, mybir.AluOpType.mult)
            nc.vector.scalar_tensor_tensor(t2[:], sci[:], 1.0, o1i[:],
                                           mybir.AluOpType.add, mybir.AluOpType.mult)
            nc.vector.tensor_sub(nr[:], nr[:], t2[:])
            nc.vector.tensor_add(nr[:], nr[:], shr[:])
            ni = bop.tile([BS, SPX], F32, tag="ni")
            t3 = bop.tile([BS, SPX], F32, tag="t3")
            nc.vector.scalar_tensor_tensor(ni[:], scr[:], 1.0, o1i[:],
                                           mybir.AluOpType.add, mybir.AluOpType.mult)
            nc.vector.scalar_tensor_tensor(t3[:], sci[:], 1.0, o1r[:],
                                           mybir.AluOpType.add, mybir.AluOpType.mult)
            nc.vector.tensor_add(ni[:], ni[:], t3[:])
            nc.vector.tensor_add(ni[:], ni[:], shi[:])
            o1rp = bop.tile([128, SPX], BF16, tag="o1rp")
            nc.scalar.activation(o1rp[0:BS, :], nr[:], AF.Relu)
            nc.vector.memset(o1rp[BS:128, :], 0.0)
            o1ip = bop.tile([128, SPX], BF16, tag="o1ip")
            nc.scalar.activation(o1ip[0:BS, :], ni[:], AF.Relu)
            nc.vector.memset(o1ip[BS:128, :], 0.0)
            for st, ln in CHUNKS:
                # o2_re
                p2r = bpp.tile([BS, 512], F32, tag="p2r")
                nc.tensor.matmul(p2r[:, :ln], w2r, o1rp[:, st:st + ln], start=True, stop=False)
                nc.tensor.matmul(p2r[:, :ln], w2in, o1ip[:, st:st + ln], start=False, stop=True)
                tr = bop.tile([BS, 512], F32, tag="tr")
                nc.scalar.activation(tr[:, :ln], p2r[:, :ln], AF.Identity, bias=b2rt[:, 0:1])
                u1 = bop.tile([BS, 512], F32, tag="u1")
                nc.scalar.activation(u1[:, :ln], tr[:, :ln], AF.Relu, bias=lam[:, 0:1])
                u2 = bop.tile([BS, 512], F32, tag="u2")
                nc.scalar.activation(u2[:, :ln], tr[:, :ln], AF.Relu, bias=lam[:, 0:1], scale=-1.0)
                o2rt = bop.tile([BS, 512], BF16, tag="o2rt")
                nc.vector.tensor_sub(o2rt[:, :ln], u1[:, :ln], u2[:, :ln])
                nc.sync.dma_start(o2r[ds(BS * b, BS), st:st + ln], o2rt[:, :ln])
                # o2_im
                p2i = bpp.tile([BS, 512], F32, tag="p2i")
                nc.tensor.matmul(p2i[:, :ln], w2r, o1ip[:, st:st + ln], start=True, stop=False)
                nc.tensor.matmul(p2i[:, :ln], w2i, o1rp[:, st:st + ln], start=False, stop=True)
                ti = bop.tile([BS, 512], F32, tag="ti")
                nc.scalar.activation(ti[:, :ln], p2i[:, :ln], AF.Identity, bias=b2it[:, 0:1])
                v1 = bop.tile([BS, 512], F32, tag="v1")
                nc.scalar.activation(v1[:, :ln], ti[:, :ln], AF.Relu, bias=lam[:, 0:1])
                v2 = bop.tile([BS, 512], F32, tag="v2")
                nc.scalar.activation(v2[:, :ln], ti[:, :ln], AF.Relu, bias=lam[:, 0:1], scale=-1.0)
                o2it = bop.tile([BS, 512], BF16, tag="o2it")
                nc.vector.tensor_sub(o2it[:, :ln], v1[:, :ln], v2[:, :ln])
                nc.sync.dma_start(o2i[ds(BS * b, BS), st:st + ln], o2it[:, :ln])
        bctx.close()
        s0.close()
    nc.compile()
    _PROG1 = nc
    return nc


# ---------------------------------------------------------------- L2 program
def _build_l2():
    global _PROG2
    if _PROG2 is not None:
        return _PROG2
    nc = bacc.Bacc("TRN2", target_bir_lowering=False, debug=False, num_devices=N_CORES)

    ma = nc.dram_tensor("ma", [EMBED, PX], F8, kind="ExternalInput")
    if USE_CC:
        mw1s = nc.dram_tensor("mw1s", [12, 6, 128, 128], F8, kind="ExternalInput")
        mw2s = nc.dram_tensor("mw2s", [6, 96, 128, 128], F8, kind="ExternalInput")
        cw1s = nc.dram_tensor("cw1s", [3, 6, 128, 128], BF16, kind="ExternalInput")
        cw2ks = nc.dram_tensor("cw2ks", [3, 6, 128, 128], BF16, kind="ExternalInput")
    else:
        mw1 = nc.dram_tensor("mw1s", [96, 6, 128, 128], F8, kind="ExternalInput")
        mw2 = nc.dram_tensor("mw2s", [48, 96, 128, 128], F8, kind="ExternalInput")
        cw1 = nc.dram_tensor("cw1s", [24, 6, 128, 128], BF16, kind="ExternalInput")
        cw2k = nc.dram_tensor("cw2ks", [24, 6, 128, 128], BF16, kind="ExternalInput")
    mb1 = nc.dram_tensor("mb1", [4 * LATENT, 1], F32, kind="ExternalInput")
    mb2 = nc.dram_tensor("mb2", [2 * LATENT, 1], F32, kind="ExternalInput")
    h2 = nc.dram_tensor("h2", [EMBED, PX], BF16, kind="ExternalInput")
    cb1 = nc.dram_tensor("cb1", [LATENT, 1], F32, kind="ExternalInput")
    cb2 = nc.dram_tensor("cb2", [EMBED, 1], F32, kind="ExternalInput")
    mlp = nc.dram_tensor("mlp", [EMBED, PX], BF16, kind="ExternalOutput")
    h1d = nc.dram_tensor("h1d", [4 * LATENT, PX], F8, kind="Internal")
    gd = nc.dram_tensor("gd", [LATENT, PX], BF16, kind="Internal")
    GROUPS = [list(range(N_CORES))]
    BYP = mybir.AluOpType.bypass

    with tile.TileContext(nc) as tc:
        # ---- stage 0: all-gather the TP-sharded weights (cuts host->device
        # transfer 8x; each core contributes 1/8 of every large weight) ----
        s0 = ExitStack()
        if not USE_CC:
            pass
        else:
         dpool = s0.enter_context(tc.tile_pool(name="wg_dram", bufs=1, space="DRAM"))
         mw1b = dpool.tile([12, 6, 128, 128], F8)
         nc.sync.dma_start(mw1b[:], mw1s[:])
         mw1 = dpool.tile([96, 6, 128, 128], F8, addr_space="Shared")
         nc.gpsimd.collective_compute("AllGather", BYP, replica_groups=GROUPS,
                                      ins=[mw1b.opt()], outs=[mw1.opt()])
         cw1b = dpool.tile([3, 6, 128, 128], BF16)
         nc.sync.dma_start(cw1b[:], cw1s[:])
         cw1 = dpool.tile([24, 6, 128, 128], BF16, addr_space="Shared")
         nc.gpsimd.collective_compute("AllGather", BYP, replica_groups=GROUPS,
                                      ins=[cw1b.opt()], outs=[cw1.opt()])
         cw2kb = dpool.tile([3, 6, 128, 128], BF16)
         nc.sync.dma_start(cw2kb[:], cw2ks[:])
         cw2k = dpool.tile([24, 6, 128, 128], BF16, addr_space="Shared")
         nc.gpsimd.collective_compute("AllGather", BYP, replica_groups=GROUPS,
                                      ins=[cw2kb.opt()], outs=[cw2k.opt()])
         mw2b = dpool.tile([6, 96, 128, 128], F8)
         nc.sync.dma_start(mw2b[:], mw2s[:])
         mw2 = dpool.tile([48, 96, 128, 128], F8, addr_space="Shared")
         nc.gpsimd.collective_compute("AllGather", BYP, replica_groups=GROUPS,
                                      ins=[mw2b.opt()], outs=[mw2.opt()])
        # ---- stage 1: conv1 (768 -> 12288), h1 to DRAM in fp8 ----
        s1 = ExitStack()
        ap = s1.enter_context(tc.tile_pool(name="m_a", bufs=1))
        wp = s1.enter_context(tc.tile_pool(name="m_w", bufs=3))
        hp = s1.enter_context(tc.tile_pool(name="m_h", bufs=4))
        bp = s1.enter_context(tc.tile_pool(name="m_b", bufs=3))
        pp = s1.enter_context(tc.tile_pool(name="m_p", bufs=4, space="PSUM"))
        mat = ap.tile([128, 6, PX], F8)
        nc.sync.dma_start(mat[:], ma.rearrange("(c p) s -> p c s", p=128))

        def conv1_body(hh):
            w1t = wp.tile([128, 6, 128], F8, tag="mw1")
            nc.sync.dma_start(w1t[:], mw1[ds(hh, 1)].rearrange("one c p m -> p (one c) m"))
            b1t = bp.tile([128, 1], F32, tag="mb1")
            nc.sync.dma_start(b1t[:], mb1[ds(hh * 128, 128), :])
            for px in range(4):
                ps = pp.tile([128, 512], F32, tag="ps1")
                for c in range(3):
                    nc.tensor.matmul(
                        ps[:], w1t[:, 2 * c:2 * c + 2, :],
                        mat[:, 2 * c:2 * c + 2, ts(px, 512)],
                        start=(c == 0), stop=(c == 2), perf_mode=DR)
                ht = hp.tile([128, 512], F8, tag="ht")
                nc.scalar.activation(ht[:], ps[:], AF.Relu, bias=b1t[:, 0:1], scale=1.0 / MS)
                nc.sync.dma_start(h1d[ds(hh * 128, 128), ts(px, 512)], ht[:])

        tc.For_i_unrolled(0, 96, 1, conv1_body, max_unroll=8)
        s1.close()

        # ---- stage 2: conv2 (12288 -> 6144) fused with fc1 + FiLM + gelu ----
        s2 = ExitStack()
        h1p = s2.enter_context(tc.tile_pool(name="c2_h1", bufs=1))
        h2p = s2.enter_context(tc.tile_pool(name="c2_h2", bufs=1))
        w2p = s2.enter_context(tc.tile_pool(name="c2_w", bufs=2))
        cwp = s2.enter_context(tc.tile_pool(name="c2_cw", bufs=2))
        stp = s2.enter_context(tc.tile_pool(name="c2_st", bufs=3))
        vp = s2.enter_context(tc.tile_pool(name="c2_v", bufs=2))
        gp = s2.enter_context(tc.tile_pool(name="c2_g", bufs=2))
        bp2 = s2.enter_context(tc.tile_pool(name="c2_b", bufs=3))
        fp2 = s2.enter_context(tc.tile_pool(name="c2_f", bufs=2))
        pp2 = s2.enter_context(tc.tile_pool(name="c2_p", bufs=2, space="PSUM"))
        for slab in range(2):
            h1b = h1p.tile([128, 96, 1024], F8, tag="h1b")
            for k in range(96):
                nc.sync.dma_start(h1b[:, k, :], h1d[ds(k * 128, 128), ts(slab, 1024)])
            h2t = h2p.tile([128, 6, 1024], BF16, tag="h2t")
            nc.sync.dma_start(h2t[:], h2[:, ts(slab, 1024)].rearrange("(c p) s -> p c s", p=128))

            def jbody(j, slab=slab, h1b=h1b, h2t=h2t):
                # fc1 strip j (bf16)
                cwt = cwp.tile([128, 6, 128], BF16, tag="cw1")
                nc.sync.dma_start(cwt[:], cw1[ds(j, 1)].rearrange("one c p m -> p (one c) m"))
                cbt = bp2.tile([128, 1], F32, tag="cb1")
                nc.sync.dma_start(cbt[:], cb1[ds(j * 128, 128), :])
                f1t = fp2.tile([128, 1024], BF16, tag="f1t")
                for half in range(2):
                    psf = pp2.tile([128, 512], F32, tag="psf")
                    for c in range(6):
                        nc.tensor.matmul(psf[:], cwt[:, c, :], h2t[:, c, ts(half, 512)],
                                         start=(c == 0), stop=(c == 5))
                    nc.scalar.activation(f1t[:, ts(half, 512)], psf[:], AF.Identity,
                                         bias=cbt[:, 0:1])
                # conv2 strips 2j (scale) and 2j+1 (shift)
                st_tiles = []
                for which in range(2):
                    w2t = w2p.tile([128, 96, 128], F8, tag="mw2")
                    nc.sync.dma_start(
                        w2t[:], mw2[ds(2 * j + which, 1)].rearrange("one k p m -> p (one k) m"))
                    b2t = bp2.tile([128, 1], F32, tag="mb2")
                    nc.sync.dma_start(b2t[:], mb2[ds(j * 256 + which * 128, 128), :])
                    psA = pp2.tile([128, 512], F32, tag="psA")
                    psB = pp2.tile([128, 512], F32, tag="psB")
                    for k in range(48):
                        nc.tensor.matmul(psA[:], w2t[:, 2 * k:2 * k + 2, :],
                                         h1b[:, 2 * k:2 * k + 2, 0:512],
                                         start=(k == 0), stop=(k == 47), perf_mode=DR)
                        nc.tensor.matmul(psB[:], w2t[:, 2 * k:2 * k + 2, :],
                                         h1b[:, 2 * k:2 * k + 2, 512:1024],
                                         start=(k == 0), stop=(k == 47), perf_mode=DR)
                    sot = stp.tile([128, 1024], BF16, tag="sst")
                    nc.scalar.activation(sot[:, 0:512], psA[:], AF.Relu,
                                         bias=b2t[:, 0:1], scale=1.0 / MS2)
                    nc.scalar.activation(sot[:, 512:1024], psB[:], AF.Relu,
                                         bias=b2t[:, 0:1], scale=1.0 / MS2)
                    st_tiles.append(sot)
                # g = gelu(f1*(1+s) + t)
                v = vp.tile([128, 1024], F32, tag="v")
                nc.vector.scalar_tensor_tensor(v[:], st_tiles[0][:], 1.0, f1t[:],
                                               mybir.AluOpType.add, mybir.AluOpType.mult)
                nc.vector.tensor_add(v[:], v[:], st_tiles[1][:])
                # exact gelu: g = v * (1 + erf(v/sqrt(2))); the 0.5 is folded
                # into the fc2 weights on the host
                et = gp.tile([128, 1024], F32, tag="et")
                nc.scalar.activation(et[:], v[:], AF.Erf, scale=0.7071067811865476)
                gt = gp.tile([128, 1024], BF16, tag="gt")
                nc.vector.scalar_tensor_tensor(gt[:], et[:], 1.0, v[:],
                                               mybir.AluOpType.add, mybir.AluOpType.mult)
                nc.sync.dma_start(gd[ds(j * 128, 128), ts(slab, 1024)], gt[:])

            tc.For_i_unrolled(0, 24, 1, jbody, max_unroll=3)
        s2.close()

        # ---- stage 3: fc2 (3072 -> 768) ----
        s3 = ExitStack()
        gkp = s3.enter_context(tc.tile_pool(name="f2_g", bufs=3))
        wkp = s3.enter_context(tc.tile_pool(name="f2_w", bufs=3))
        obp = s3.enter_context(tc.tile_pool(name="f2_b", bufs=1))
        otp = s3.enter_context(tc.tile_pool(name="f2_o", bufs=3))
        pp3 = s3.enter_context(tc.tile_pool(name="f2_p", bufs=1, space="PSUM"))
        cb2t = obp.tile([128, 6, 1], F32)
        nc.sync.dma_start(cb2t[:], cb2.rearrange("(o p) one -> p o one", p=128))
        for halfq in range(4):
            pos = [pp3.tile([128, 512], F32, tag=f"po{o}", name=f"po{o}") for o in range(6)]
            for k in range(24):
                gkt = gkp.tile([128, 512], BF16, tag="gk")
                nc.sync.dma_start(gkt[:], gd[ds(k * 128, 128), ts(halfq, 512)])
                wkt = wkp.tile([128, 6, 128], BF16, tag="wk")
                nc.sync.dma_start(wkt[:], cw2k[ds(k, 1)].rearrange("one o p m -> p (one o) m"))
                for o in range(6):
                    nc.tensor.matmul(pos[o][:], wkt[:, o, :], gkt[:],
                                     start=(k == 0), stop=(k == 23))
            for o in range(6):
                ot = otp.tile([128, 512], BF16, tag="ot")
                nc.scalar.activation(ot[:], pos[o][:], AF.Identity, bias=cb2t[:, o, 0:1])
                nc.sync.dma_start(mlp[ds(o * 128, 128), ts(halfq, 512)], ot[:])
        s3.close()
        s0.close()
    nc.compile()
    _PROG2 = nc
    return nc


# ---------------------------------------------------------------- host side
def _bf16(x):
    return np.ascontiguousarray(np.asarray(x, np.float32)).astype(ml_dtypes.bfloat16)


def _f8(x, s=1.0):
    return np.ascontiguousarray(np.asarray(x, np.float32) * s).astype(ml_dtypes.float8_e4m3)


def _layernorm(x, w, b):
    m = x.mean(-1, keepdims=True)
    v = x.var(-1, keepdims=True)
    return (x - m) / np.sqrt(v + EPS) * w + b


def _prep_weights(w1, b1, w2, b2, f_c1_w, f_c1_b, f_c2_w, f_c2_b,
                  fc1_w, fc1_b, fc2_w, fc2_b, m_c1_w, m_c1_b, m_c2_w, m_c2_b):
    key = (id(m_c2_w), id(fc1_w), id(f_c2_w), id(w1))
    if _WCACHE.get("key") == key:
        return _WCACHE["val"]
    f32 = np.float32

    def FDTC(a, s=1.0):
        return _f8(a, s) if FP8_F else _bf16(np.asarray(a, f32) * s)

    # L1 weights (prescaled by MS/MS2; device divides back via activation scale)
    fw1 = FDTC(np.asarray(f_c1_w, f32).T.reshape(6, 128, 24, 128).transpose(2, 0, 1, 3), MS)
    fw2 = FDTC(np.asarray(f_c2_w, f32).T.reshape(24, 128, 12, 128).transpose(0, 2, 1, 3), MS2)
    fb1 = np.asarray(f_c1_b, f32).reshape(-1, 1)
    fb2 = np.asarray(f_c2_b, f32).reshape(-1, 1)
    w1_ = np.asarray(w1, f32)
    w2_ = np.asarray(w2, f32)
    bw_stack = np.stack(
        [w1_[0], w1_[1], -w1_[1], w2_[0], w2_[1], -w2_[1]], axis=1)  # [8,6,96,96]
    bw_pad = np.zeros((BLOCKS, 6, 128, BS), np.float32)
    bw_pad[:, :, :BS, :] = bw_stack
    bwp = _bf16(bw_pad)
    b1_ = np.asarray(b1, f32)
    b2_ = np.asarray(b2, f32)
    l1 = [{
        "fw1s": np.ascontiguousarray(fw1[3 * k:3 * k + 3]) if USE_CC else fw1,
        "fw2s": np.ascontiguousarray(fw2[3 * k:3 * k + 3]) if USE_CC else fw2,
        "fb1": fb1, "fb2": fb2, "bw": bwp,
        "bb1r": b1_[0].reshape(-1, 1), "bb1i": b1_[1].reshape(-1, 1),
        "bb2r": b2_[0].reshape(-1, 1), "bb2i": b2_[1].reshape(-1, 1),
    } for k in range(N_CORES)]

    # L2 weights
    mw1 = _f8(np.asarray(m_c1_w, f32).T.reshape(6, 128, 96, 128).transpose(2, 0, 1, 3), MS)
    perm = np.empty(2 * LATENT, np.int64)
    for j in range(24):
        perm[256 * j:256 * j + 128] = np.arange(128 * j, 128 * j + 128)
        perm[256 * j + 128:256 * j + 256] = np.arange(LATENT + 128 * j, LATENT + 128 * j + 128)
    m2p = np.asarray(m_c2_w, f32)[perm]
    mw2 = _f8(m2p.T.reshape(96, 128, 48, 128).transpose(2, 0, 1, 3), MS2)
    mb2 = np.asarray(m_c2_b, f32)[perm].reshape(-1, 1)
    cw1 = _bf16(np.asarray(fc1_w, f32).T.reshape(6, 128, 24, 128).transpose(2, 0, 1, 3))
    # 0.5 of the exact gelu is folded here (device computes v*(1+erf(v/sqrt2)))
    cw2k = _bf16(0.5 * np.asarray(fc2_w, f32).T.reshape(24, 128, 6, 128).transpose(0, 2, 1, 3))
    l2 = [{
        "mw1s": np.ascontiguousarray(mw1[12 * k:12 * k + 12]) if USE_CC else mw1,
        "mw2s": np.ascontiguousarray(mw2[6 * k:6 * k + 6]) if USE_CC else mw2,
        "cw1s": np.ascontiguousarray(cw1[3 * k:3 * k + 3]) if USE_CC else cw1,
        "cw2ks": np.ascontiguousarray(cw2k[3 * k:3 * k + 3]) if USE_CC else cw2k,
        "mb1": np.asarray(m_c1_b, f32).reshape(-1, 1), "mb2": mb2,
        "cb1": np.asarray(fc1_b, f32).reshape(-1, 1),
        "cb2": np.asarray(fc2_b, f32).reshape(-1, 1),
    } for k in range(N_CORES)]
    _WCACHE["key"] = key
    _WCACHE["val"] = (l1, l2)
    return l1, l2


def kernel(x, mod_embed, norm1_w, norm1_b, norm2_w, norm2_b, w1, b1, w2, b2,
           f_c1_w, f_c1_b, f_c2_w, f_c2_b, fc1_w, fc1_b, fc2_w, fc2_b,
           m_c1_w, m_c1_b, m_c2_w, m_c2_b):
    global LAST_RESULTS
    f32 = np.float32
    x = np.asarray(x, f32)
    assert x.shape == (1, H, W, EMBED)
    x2 = x[0].reshape(H * W, EMBED)
    mod2 = np.asarray(mod_embed, f32)[0].reshape(H * W, EMBED)

    trace = bool(os.environ.get("KERNEL_TRACE"))
    l1w, l2w = _prep_weights(w1, b1, w2, b2, f_c1_w, f_c1_b, f_c2_w, f_c2_b,
                             fc1_w, fc1_b, fc2_w, fc2_b, m_c1_w, m_c1_b, m_c2_w, m_c2_b)

    # host: LN1 + forward FFTs
    import scipy.fft as sfft
    xn2 = _layernorm(x2, np.asarray(norm1_w, f32), np.asarray(norm1_b, f32))
    xf = sfft.rfft2(xn2.reshape(H, W, EMBED), axes=(0, 1), norm="ortho", workers=8)
    mf = sfft.rfft2(mod2.reshape(H, W, EMBED), axes=(0, 1), norm="ortho", workers=8)
    xr_f = np.ascontiguousarray(xf.real.astype(f32)).reshape(SPEC, EMBED)
    xi_f = np.ascontiguousarray(xf.imag.astype(f32)).reshape(SPEC, EMBED)
    mr_f = np.ascontiguousarray(mf.real.astype(f32)).reshape(SPEC, EMBED)
    mi_f = np.ascontiguousarray(mf.imag.astype(f32)).reshape(SPEC, EMBED)

    # ---- launch 1 ----
    nc1 = _build_l1()
    FDTC = (lambda a: _f8(a)) if FP8_F else _bf16
    in1 = []
    for k in range(N_CORES):
        sl = slice(SPX * k, SPX * (k + 1))
        m = dict(l1w[k])
        m["fa"] = FDTC(np.concatenate([mr_f[sl].T, mi_f[sl].T], axis=1))
        xr_b = np.zeros((BLOCKS, 128, SPX), np.float32)
        xr_b[:, :BS] = xr_f[sl].T.reshape(BLOCKS, BS, SPX)
        xi_b = np.zeros((BLOCKS, 128, SPX), np.float32)
        xi_b[:, :BS] = xi_f[sl].T.reshape(BLOCKS, BS, SPX)
        m["xr"] = _bf16(xr_b)
        m["xi"] = _bf16(xi_b)
        in1.append(m)
    res1 = run_bass_kernel_spmd(nc1, in1, core_ids=list(range(N_CORES)), trace=trace)

    spec = np.empty((SPEC, EMBED), np.complex64)
    for k in range(N_CORES):
        sl = slice(SPX * k, SPX * (k + 1))
        spec[sl] = (res1.results[k]["o2r"].astype(f32).T
                    + 1j * res1.results[k]["o2i"].astype(f32).T)
    filt = sfft.irfft2(spec.reshape(H, WF, EMBED), s=(H, W), axes=(0, 1),
                       norm="ortho", workers=8).astype(f32).reshape(H * W, EMBED)
    h_mid = filt + xn2 + x2
    h22 = _layernorm(h_mid, np.asarray(norm2_w, f32), np.asarray(norm2_b, f32))

    # ---- launch 2 ----
    nc2 = _build_l2()
    in2 = []
    for k in range(N_CORES):
        sl = slice(PX * k, PX * (k + 1))
        m = dict(l2w[k])
        m["ma"] = _f8(mod2[sl].T)
        m["h2"] = _bf16(h22[sl].T)
        in2.append(m)
    res2 = run_bass_kernel_spmd(nc2, in2, core_ids=list(range(N_CORES)), trace=trace)
    LAST_RESULTS = (res1, res2)

    out = np.empty((H * W, EMBED), f32)
    for k in range(N_CORES):
        sl = slice(PX * k, PX * (k + 1))
        out[sl] = res2.results[k]["mlp"].astype(f32).T + h_mid[sl]
    return out.reshape(1, H, W, EMBED).astype(f32)
